# revision 8
# baseline (speedup 1.0000x reference)
"""Multi-head attention + output dense + LayerNorm + residual, on 8 NeuronCores.

Sharding: core c -> (batch b = c//2, query-half hf = c%2). Each core runs the
full 16-head attention for its 1024 queries against its batch's full 2048
keys (K/V projections are recomputed per query-half; no collectives needed).
The host reorders tokens so each core's queries are always rows 0:1024 of its
input slab -- key order is softmax-invariant as long as the mask is permuted
identically, so the device program is completely SPMD-uniform.

v2 layout choices (vs v1):
  - The additive attention mask is folded multiplicatively into V:
    softmax(S + m)_k = exp(S_k) w_k / sum_j exp(S_j) w_j with w = exp(m).
    V rows are scaled by w and the denominator column of V holds w, so the
    score matmuls have a pure 64-deep contraction (no mask row).
  - Score matmuls for the two heads of a pair run CONCURRENTLY on the PE
    array via 64x128 row tiling (tile_position (0,0)/(64,0)): K^T/Q^T for
    head-even live on SBUF partitions 0:64, head-odd on 64:128. This halves
    PE time for scores.
  - One exp activation per key chunk covers both heads ([128,2,512] PSUM ->
    bf16), with deep es buffering so the V-build phase overlaps head-pair
    0's softmax on ScalarE.
  - ctx matmuls keep the 65-column V (65th column = w) so row 64 of the
    accumulated ctx is the softmax denominator for free.
  - Phase 3 evacuates the out-proj PSUM via ScalarE copy (idle in the tail),
    runs bn_stats directly on PSUM, and puts residual adds on GpSimd.
"""

import numpy as np

B, S, H, NH = 4, 2048, 1024, 16
HD = H // NH  # 64
SQ = S // 2  # queries per core
NCORES = 8
NPAIR = NH // 2  # head pairs
NCI = H // 128  # 8 contraction chunks
NKC = S // 128  # 16 key chunks
EPS = 1e-12

_cache = {}


def _build():
    import concourse.bass as bass
    import concourse.bacc as bacc
    import concourse.mybir as mybir
    import concourse.tile as tile

    fp32 = mybir.dt.float32
    bf16 = mybir.dt.bfloat16
    AF = mybir.ActivationFunctionType
    OP = mybir.AluOpType

    nc = bacc.Bacc("TRN2", target_bir_lowering=False, debug=False)

    xkv = nc.dram_tensor("xkv", [S, H], fp32, kind="ExternalInput").ap()
    xtb_d = nc.dram_tensor("xtb", [NKC, 128, NCI, 128], bf16, kind="ExternalInput").ap()
    wexp32_d = nc.dram_tensor("wexp32", [S], fp32, kind="ExternalInput").ap()
    wexpbf_d = nc.dram_tensor("wexpbf", [S], bf16, kind="ExternalInput").ap()
    wq_d = nc.dram_tensor("wq", [H, H], bf16, kind="ExternalInput").ap()
    wk_d = nc.dram_tensor("wk", [H, H], bf16, kind="ExternalInput").ap()
    wv_d = nc.dram_tensor("wv", [H, H], bf16, kind="ExternalInput").ap()
    wd_d = nc.dram_tensor("wd", [H, H], bf16, kind="ExternalInput").ap()
    bq_d = nc.dram_tensor("bq", [H], fp32, kind="ExternalInput").ap()
    bk_d = nc.dram_tensor("bk", [H], fp32, kind="ExternalInput").ap()
    bv_d = nc.dram_tensor("bv", [H], bf16, kind="ExternalInput").ap()
    bd_d = nc.dram_tensor("bd", [H], bf16, kind="ExternalInput").ap()
    gamma_d = nc.dram_tensor("gamma", [H], fp32, kind="ExternalInput").ap()
    beta_d = nc.dram_tensor("beta", [H], fp32, kind="ExternalInput").ap()
    out_d = nc.dram_tensor("out", [SQ, H], fp32, kind="ExternalOutput").ap()

    def bcast128(ap):
        return bass.AP(tensor=ap.tensor, offset=ap.offset, ap=[[0, 128]] + list(ap.ap))

    def row1(ap):
        return bass.AP(tensor=ap.tensor, offset=ap.offset, ap=[[0, 1]] + list(ap.ap))

    with tile.TileContext(nc) as tc:
        with (
            tc.tile_pool(name="consts", bufs=1) as consts,
            tc.tile_pool(name="ctxT", bufs=1) as ctxt_pool,
        ):
            # --- constants ---
            bqT = consts.tile([128, NCI], fp32)
            nc.sync.dma_start(out=bqT, in_=bq_d.rearrange("(c p) -> p c", p=128))
            bkT = consts.tile([128, NCI], fp32)
            nc.sync.dma_start(out=bkT, in_=bk_d.rearrange("(c p) -> p c", p=128))
            wexp_sb = consts.tile([128, NKC], fp32)
            nc.sync.dma_start(out=wexp_sb, in_=wexp32_d.rearrange("(c p) -> p c", p=128))
            gamma_b = consts.tile([128, H], fp32)
            nc.sync.dma_start(out=gamma_b, in_=bcast128(gamma_d))
            beta_b = consts.tile([128, H], fp32)
            nc.sync.dma_start(out=beta_b, in_=bcast128(beta_d))
            eps_sb = consts.tile([128, 1], fp32)
            nc.vector.memset(eps_sb, EPS)
            sel65 = consts.tile([65, 128], bf16)
            nc.vector.memset(sel65, 0.0)
            nc.vector.memset(sel65[0:1, 0:64], 1.0)
            nc.vector.memset(sel65[64:65, 64:128], 1.0)
            recip_bf = consts.tile([65, 512], bf16)
            nc.vector.memset(recip_bf, 0.0)
            ones1 = consts.tile([1, 128], bf16)
            nc.vector.memset(ones1, 1.0)
            bv_row = consts.tile([1, H], bf16)
            nc.gpsimd.dma_start(out=bv_row, in_=row1(bv_d))
            bd_row = consts.tile([1, H], bf16)
            nc.gpsimd.dma_start(out=bd_row, in_=row1(bd_d))

            # ctxT[hl*64+d, hp, q] = ctx[q, (hp*2+hl)*64+d] / den
            ctxt = ctxt_pool.tile([128, NPAIR, SQ], bf16)
            wd_sb = ctxt_pool.tile([128, NCI, H], bf16, name="wd_sb")

            ctx_mid = tc.tile_pool(name="midA", bufs=1)
            midA = ctx_mid.__enter__()
            xt = midA.tile([128, NCI, S], bf16, name="xt")
            wq_full = midA.tile([128, NCI, H], bf16, name="wq_full")
            wk_full = midA.tile([128, NCI, H], bf16, name="wk_full")
            wv_full = midA.tile([128, NCI, H], bf16, name="wv_full")
            v_all = midA.tile([128, NKC, NH, 65], bf16, name="v_all")

            # input DMAs: what head-pair 0's projections need comes first
            nc.sync.dma_start(out=wq_full, in_=wq_d.rearrange("(c p) n -> p c n", p=128))
            for tch in range(NKC):
                nc.sync.dma_start(out=xt[:, :, tch * 128 : (tch + 1) * 128], in_=xtb_d[tch])
            nc.sync.dma_start(out=wk_full, in_=wk_d.rearrange("(c p) n -> p c n", p=128))
            nc.sync.dma_start(out=wv_full, in_=wv_d.rearrange("(c p) n -> p c n", p=128))
            nc.sync.dma_start(out=wd_sb, in_=wd_d.rearrange("(c p) n -> p c n", p=128))
            # w column of V: v_all[t, kc, h, 64] = w[kc*128+t] for every head
            for tb in range(NKC):
                nc.gpsimd.dma_start(
                    out=v_all[:, tb, :, 64:65],
                    in_=bass.AP(
                        tensor=wexpbf_d.tensor,
                        offset=wexpbf_d.offset + tb * 128,
                        ap=[[1, 128], [0, NH], [0, 1]],
                    ),
                )

            # --- phase 2: per head-pair projections + attention ---
            # (V build for all heads is emitted inside the hp==0 iteration so it
            #  overlaps head-pair 0's softmax on ScalarE.)
            with (
                tc.tile_pool(name="pairbuf", bufs=2) as pairbuf,
                tc.tile_pool(name="exps", bufs=12) as exps_pool,
                tc.tile_pool(name="sums", bufs=1) as sums_pool,
                tc.tile_pool(name="pp", bufs=2, space="PSUM") as pp,
                tc.tile_pool(name="sp", bufs=2, space="PSUM") as sp,
                tc.tile_pool(name="cp", bufs=2, space="PSUM") as cp,
            ):
                # V build for all heads (own phase, 128x128 mode only)
                for tb in range(NKC):
                    for nh in range(2):
                        pv = pp.tile([128, 512], fp32, tag="proj")
                        for ci in range(NCI):
                            nc.tensor.matmul(
                                pv,
                                xt[:, ci, tb * 128 : (tb + 1) * 128],
                                wv_full[:, ci, nh * 512 : (nh + 1) * 512],
                                start=(ci == 0),
                                stop=False,
                            )
                        nc.tensor.matmul(
                            pv,
                            ones1,
                            bv_row[:, nh * 512 : (nh + 1) * 512],
                            start=False,
                            stop=True,
                        )
                        # scale by w while evacuating PSUM
                        nc.vector.tensor_scalar_mul(
                            out=v_all[:, tb, nh * 8 : (nh + 1) * 8, 0:64],
                            in0=pv.rearrange("p (a b) -> p a b", a=8),
                            scalar1=wexp_sb[:, tb : tb + 1],
                        )
                for hp in range(NPAIR):
                    cols = slice(hp * 128, (hp + 1) * 128)
                    # Q'^T for the pair: [128, SQ], partitions 0:64 head-even,
                    # 64:128 head-odd
                    qt = pairbuf.tile([128, SQ], bf16, tag="qt", name=f"qt{hp}")
                    for qb in range(SQ // 512):
                        pq = pp.tile([128, 512], fp32, tag="proj")
                        for ci in range(NCI):
                            nc.tensor.matmul(
                                pq,
                                wq_full[:, ci, cols],
                                xt[:, ci, qb * 512 : (qb + 1) * 512],
                                start=(ci == 0),
                                stop=(ci == NCI - 1),
                            )
                        nc.vector.tensor_scalar_add(
                            out=qt[:, qb * 512 : (qb + 1) * 512],
                            in0=pq,
                            scalar1=bqT[:, hp : hp + 1],
                        )

                    # K'^T for the pair: [128, S]
                    kt = pairbuf.tile([128, S], bf16, tag="kt", name=f"kt{hp}")
                    for tb in range(S // 512):
                        pk = pp.tile([128, 512], fp32, tag="proj")
                        for ci in range(NCI):
                            nc.tensor.matmul(
                                pk,
                                wk_full[:, ci, cols],
                                xt[:, ci, tb * 512 : (tb + 1) * 512],
                                start=(ci == 0),
                                stop=(ci == NCI - 1),
                            )
                        nc.vector.tensor_scalar_add(
                            out=kt[:, tb * 512 : (tb + 1) * 512],
                            in0=pk,
                            scalar1=bkT[:, hp : hp + 1],
                        )

                    # attention for this pair
                    for qb in range(SQ // 512):
                        qsl = slice(qb * 512, (qb + 1) * 512)
                        # scores + exp, per key chunk, both heads at once
                        es_list = []
                        for kc in range(NKC):
                            ksl = slice(kc * 128, (kc + 1) * 128)
                            ps2 = sp.tile([128, 2, 512], fp32)
                            for hl in range(2):
                                hsl = slice(hl * 64, (hl + 1) * 64)
                                nc.tensor.matmul(
                                    ps2[:, hl, :],
                                    kt[hsl, ksl],
                                    qt[hsl, qsl],
                                    start=True,
                                    stop=True,
                                )
                            es2 = exps_pool.tile([128, 2, 512], bf16)
                            nc.scalar.activation(
                                out=es2.rearrange("p a b -> p (a b)"),
                                in_=ps2.rearrange("p a b -> p (a b)"),
                                func=AF.Exp,
                                scale=0.125,
                            )
                            es_list.append(es2)


                        # ctx accumulation per head
                        pc = [
                            cp.tile([65, 512], fp32, tag=f"pc{hl}", name=f"pc{hl}", bufs=1)
                            for hl in range(2)
                        ]
                        for kc in range(NKC):
                            for hl in range(2):
                                nc.tensor.matmul(
                                    pc[hl],
                                    v_all[:, kc, hp * 2 + hl, :],
                                    es_list[kc][:, hl, :],
                                    start=(kc == 0),
                                    stop=(kc == NKC - 1),
                                )

                        # normalize: ctxt[:, hp, qsl] = ctx^T / den
                        for hl in range(2):
                            hsl = slice(hl * 64, (hl + 1) * 64)
                            nc.vector.tensor_copy(out=ctxt[hsl, hp, qsl], in_=pc[hl][0:64, :])
                        pb = pp.tile([128, 512], fp32, tag="proj")
                        for hl in range(2):
                            sums1 = sums_pool.tile(
                                [1, 512], fp32, tag=f"sums{hl}", name=f"sums{hl}"
                            )
                            nc.vector.tensor_copy(out=sums1, in_=pc[hl][64:65, :])
                            recip1 = sums_pool.tile([1, 512], fp32, tag=f"recip{hl}")
                            nc.vector.reciprocal_approx_fast(out=recip1, in_=sums1)
                            nc.vector.tensor_copy(
                                out=recip_bf[hl * 64 : hl * 64 + 1, :], in_=recip1
                            )
                        nc.tensor.matmul(pb, sel65, recip_bf, start=True, stop=True)
                        nc.vector.tensor_mul(ctxt[:, hp, qsl], ctxt[:, hp, qsl], pb)

            ctx_mid.__exit__(None, None, None)

            # --- phase 3: output projection + LayerNorm + residual ---
            with (
                tc.tile_pool(name="hid", bufs=3) as hid_pool,
                tc.tile_pool(name="lnbuf", bufs=3) as lnbuf,
                tc.tile_pool(name="op", bufs=3, space="PSUM") as op_pool,
            ):
                for qt_ in range(SQ // 128):
                    qsl = slice(qt_ * 128, (qt_ + 1) * 128)
                    hid = hid_pool.tile([128, H], fp32)
                    stats = lnbuf.tile([128, 2, 6], fp32, tag="stats")
                    for nb in range(2):
                        po = op_pool.tile([128, 512], fp32)
                        for ci in range(NCI):
                            nc.tensor.matmul(
                                po,
                                ctxt[:, ci, qsl],
                                wd_sb[:, ci, nb * 512 : (nb + 1) * 512],
                                start=(ci == 0),
                                stop=False,
                            )
                        nc.tensor.matmul(
                            po,
                            ones1,
                            bd_row[:, nb * 512 : (nb + 1) * 512],
                            start=False,
                            stop=True,
                        )
                        # LayerNorm stats straight off PSUM; ScalarE evacuates
                        nc.vector.bn_stats(out=stats[:, nb, :], in_=po)
                        nc.scalar.copy(out=hid[:, nb * 512 : (nb + 1) * 512], in_=po)
                    mv = lnbuf.tile([128, 2], fp32, tag="mv")
                    nc.vector.bn_aggr(out=mv, in_=stats)
                    rstd = lnbuf.tile([128, 1], fp32, tag="rstd")
                    nc.scalar.activation(
                        out=rstd, in_=mv[:, 1:2], func=AF.Sqrt, bias=eps_sb
                    )
                    nc.vector.reciprocal(rstd, rstd)
                    # residual + beta (overlaps with stats)
                    x_res = lnbuf.tile([128, H], fp32, tag="xres")
                    nc.sync.dma_start(out=x_res, in_=xkv[qsl, :])
                    xbeta = lnbuf.tile([128, H], fp32, tag="xbeta")
                    nc.gpsimd.tensor_tensor(out=xbeta, in0=x_res, in1=beta_b, op=OP.add)
                    # (hid - mu) * rstd * gamma + (x + beta)
                    norm = lnbuf.tile([128, H], fp32, tag="norm")
                    nc.vector.tensor_scalar(
                        out=norm,
                        in0=hid,
                        scalar1=mv[:, 0:1],
                        scalar2=rstd,
                        op0=OP.subtract,
                        op1=OP.mult,
                    )
                    nc.vector.tensor_mul(norm, norm, gamma_b)
                    final = lnbuf.tile([128, H], fp32, tag="final")
                    nc.gpsimd.tensor_tensor(out=final, in0=norm, in1=xbeta, op=OP.add)
                    nc.sync.dma_start(out=out_d[qsl, :], in_=final)

    nc.compile()
    return nc


def get_nc():
    if "nc" not in _cache:
        _cache["nc"] = _build()
    return _cache["nc"]


def make_in_maps(inputs):
    q = np.ascontiguousarray(np.asarray(inputs["query"], dtype=np.float32))
    am = np.asarray(inputs["attention_mask"], dtype=np.float32).reshape(B, S)
    import ml_dtypes

    bfl = ml_dtypes.bfloat16
    shared = {
        "wq": np.ascontiguousarray(np.asarray(inputs["Wq"], np.float32).astype(bfl)),
        "wk": np.ascontiguousarray(np.asarray(inputs["Wk"], np.float32).astype(bfl)),
        "wv": np.ascontiguousarray(np.asarray(inputs["Wv"], np.float32).astype(bfl)),
        "wd": np.ascontiguousarray(np.asarray(inputs["Wd"], np.float32).astype(bfl)),
        "bq": np.asarray(inputs["bq"], np.float32),
        "bk": np.asarray(inputs["bk"], np.float32),
        "bv": np.asarray(inputs["bv"], np.float32).astype(bfl),
        "bd": np.asarray(inputs["bd"], np.float32).astype(bfl),
        "gamma": np.asarray(inputs["ln_gamma"], np.float32),
        "beta": np.asarray(inputs["ln_beta"], np.float32),
    }
    in_maps = []
    for c in range(NCORES):
        b, hf = c // 2, c % 2
        # queries first, then the other half -- key order is softmax-invariant
        if hf == 0:
            xkv = q[b]
            mask = am[b]
        else:
            xkv = np.concatenate([q[b, SQ:], q[b, :SQ]], axis=0)
            mask = np.concatenate([am[b, SQ:], am[b, :SQ]], axis=0)
        m = dict(shared)
        m["xkv"] = np.ascontiguousarray(xkv)
        xtc = xkv.reshape(S // 128, 128, H // 128, 128).transpose(0, 3, 2, 1)
        m["xtb"] = np.ascontiguousarray(xtc.astype(bfl))
        wexp = np.exp(mask).astype(np.float32)
        m["wexp32"] = np.ascontiguousarray(wexp)
        m["wexpbf"] = np.ascontiguousarray(wexp.astype(bfl))
        in_maps.append(m)
    return in_maps


def assemble(results):
    out = np.empty((B, S, H), dtype=np.float32)
    for c in range(NCORES):
        b, hf = c // 2, c % 2
        out[b, hf * SQ : (hf + 1) * SQ, :] = results[c]["out"]
    return out


def kernel(**inputs):
    from concourse.bass_utils import run_bass_kernel_spmd

    nc = get_nc()
    in_maps = make_in_maps(inputs)
    res = run_bass_kernel_spmd(nc, in_maps, core_ids=list(range(NCORES)))
    return assemble(res.results)


if __name__ == "__main__":
    rng = np.random.default_rng(0)
    inputs = {
        "query": rng.standard_normal((B, S, H), dtype=np.float32),
        "attention_mask": np.zeros((B, 1, 1, S), np.float32),
        "Wq": rng.standard_normal((H, H), dtype=np.float32) * 0.02,
        "bq": np.zeros(H, np.float32),
        "Wk": rng.standard_normal((H, H), dtype=np.float32) * 0.02,
        "bk": np.zeros(H, np.float32),
        "Wv": rng.standard_normal((H, H), dtype=np.float32) * 0.02,
        "bv": np.zeros(H, np.float32),
        "Wd": rng.standard_normal((H, H), dtype=np.float32) * 0.02,
        "bd": np.zeros(H, np.float32),
        "ln_gamma": np.ones(H, np.float32),
        "ln_beta": np.zeros(H, np.float32),
    }
    out = kernel(**inputs)
    print(out.shape, out.dtype)


# revision 10
# speedup vs baseline: 1.0033x; 1.0033x over previous
"""Multi-head attention + output dense + LayerNorm + residual, on 8 NeuronCores.

Sharding: core c -> (batch b = c//2, query-half hf = c%2). Each core runs the
full 16-head attention for its 1024 queries against its batch's full 2048
keys (K/V projections are recomputed per query-half; no collectives needed).
The host reorders tokens so each core's queries are always rows 0:1024 of its
input slab -- key order is softmax-invariant as long as the mask is permuted
identically, so the device program is completely SPMD-uniform.

v2 layout choices (vs v1):
  - The additive attention mask is folded multiplicatively into V:
    softmax(S + m)_k = exp(S_k) w_k / sum_j exp(S_j) w_j with w = exp(m).
    V rows are scaled by w and the denominator column of V holds w, so the
    score matmuls have a pure 64-deep contraction (no mask row).
  - Score matmuls for the two heads of a pair run CONCURRENTLY on the PE
    array via 64x128 row tiling (tile_position (0,0)/(64,0)): K^T/Q^T for
    head-even live on SBUF partitions 0:64, head-odd on 64:128. This halves
    PE time for scores.
  - One exp activation per key chunk covers both heads ([128,2,512] PSUM ->
    bf16), with deep es buffering so the V-build phase overlaps head-pair
    0's softmax on ScalarE.
  - ctx matmuls keep the 65-column V (65th column = w) so row 64 of the
    accumulated ctx is the softmax denominator for free.
  - Phase 3 evacuates the out-proj PSUM via ScalarE copy (idle in the tail),
    runs bn_stats directly on PSUM, and puts residual adds on GpSimd.
"""

import numpy as np

B, S, H, NH = 4, 2048, 1024, 16
HD = H // NH  # 64
SQ = S // 2  # queries per core
NCORES = 8
NPAIR = NH // 2  # head pairs
NCI = H // 128  # 8 contraction chunks
NKC = S // 128  # 16 key chunks
EPS = 1e-12

_cache = {}


def _build():
    import concourse.bass as bass
    import concourse.bacc as bacc
    import concourse.mybir as mybir
    import concourse.tile as tile

    fp32 = mybir.dt.float32
    bf16 = mybir.dt.bfloat16
    AF = mybir.ActivationFunctionType
    OP = mybir.AluOpType

    nc = bacc.Bacc("TRN2", target_bir_lowering=False, debug=False)

    xkv = nc.dram_tensor("xkv", [S, H], fp32, kind="ExternalInput").ap()
    xtb_d = nc.dram_tensor("xtb", [NKC, 128, NCI, 128], bf16, kind="ExternalInput").ap()
    wexp32_d = nc.dram_tensor("wexp32", [S], fp32, kind="ExternalInput").ap()
    wexpbf_d = nc.dram_tensor("wexpbf", [S], bf16, kind="ExternalInput").ap()
    wq_d = nc.dram_tensor("wq", [H, H], bf16, kind="ExternalInput").ap()
    wk_d = nc.dram_tensor("wk", [H, H], bf16, kind="ExternalInput").ap()
    wv_d = nc.dram_tensor("wv", [H, H], bf16, kind="ExternalInput").ap()
    wd_d = nc.dram_tensor("wd", [H, H], bf16, kind="ExternalInput").ap()
    bq_d = nc.dram_tensor("bq", [H], fp32, kind="ExternalInput").ap()
    bk_d = nc.dram_tensor("bk", [H], fp32, kind="ExternalInput").ap()
    bv_d = nc.dram_tensor("bv", [H], bf16, kind="ExternalInput").ap()
    bd_d = nc.dram_tensor("bd", [H], bf16, kind="ExternalInput").ap()
    gamma_d = nc.dram_tensor("gamma", [H], fp32, kind="ExternalInput").ap()
    beta_d = nc.dram_tensor("beta", [H], fp32, kind="ExternalInput").ap()
    out_d = nc.dram_tensor("out", [SQ, H], fp32, kind="ExternalOutput").ap()

    def bcast128(ap):
        return bass.AP(tensor=ap.tensor, offset=ap.offset, ap=[[0, 128]] + list(ap.ap))

    def row1(ap):
        return bass.AP(tensor=ap.tensor, offset=ap.offset, ap=[[0, 1]] + list(ap.ap))

    with tile.TileContext(nc) as tc:
        with (
            tc.tile_pool(name="consts", bufs=1) as consts,
            tc.tile_pool(name="ctxT", bufs=1) as ctxt_pool,
        ):
            # --- constants ---
            bqT = consts.tile([128, NCI], fp32)
            nc.sync.dma_start(out=bqT, in_=bq_d.rearrange("(c p) -> p c", p=128))
            bkT = consts.tile([128, NCI], fp32)
            nc.sync.dma_start(out=bkT, in_=bk_d.rearrange("(c p) -> p c", p=128))
            wexp_sb = consts.tile([128, NKC], fp32)
            nc.sync.dma_start(out=wexp_sb, in_=wexp32_d.rearrange("(c p) -> p c", p=128))
            gamma_b = consts.tile([128, H], fp32)
            nc.sync.dma_start(out=gamma_b, in_=bcast128(gamma_d))
            beta_b = consts.tile([128, H], fp32)
            nc.sync.dma_start(out=beta_b, in_=bcast128(beta_d))
            eps_sb = consts.tile([128, 1], fp32)
            nc.vector.memset(eps_sb, EPS)
            sel65 = consts.tile([65, 128], bf16)
            nc.vector.memset(sel65, 0.0)
            nc.vector.memset(sel65[0:1, 0:64], 1.0)
            nc.vector.memset(sel65[64:65, 64:128], 1.0)
            recip_bf = consts.tile([65, 512], bf16)
            nc.vector.memset(recip_bf, 0.0)
            ones1 = consts.tile([1, 128], bf16)
            nc.vector.memset(ones1, 1.0)
            bv_row = consts.tile([1, H], bf16)
            nc.gpsimd.dma_start(out=bv_row, in_=row1(bv_d))
            bd_row = consts.tile([1, H], bf16)
            nc.gpsimd.dma_start(out=bd_row, in_=row1(bd_d))

            # ctxT[hl*64+d, hp, q] = ctx[q, (hp*2+hl)*64+d] / den
            ctxt = ctxt_pool.tile([128, NPAIR, SQ], bf16)
            wd_sb = ctxt_pool.tile([128, NCI, H], bf16, name="wd_sb")

            ctx_mid = tc.tile_pool(name="midA", bufs=1)
            midA = ctx_mid.__enter__()
            xt = midA.tile([128, NCI, S], bf16, name="xt")
            wq_full = midA.tile([128, NCI, H], bf16, name="wq_full")
            wk_full = midA.tile([128, NCI, H], bf16, name="wk_full")
            wv_full = midA.tile([128, NCI, H], bf16, name="wv_full")
            v_all = midA.tile([128, NKC, NH, 65], bf16, name="v_all")

            # input DMAs: what head-pair 0's projections need comes first
            nc.sync.dma_start(out=wq_full, in_=wq_d.rearrange("(c p) n -> p c n", p=128))
            for tch in range(NKC):
                nc.sync.dma_start(out=xt[:, :, tch * 128 : (tch + 1) * 128], in_=xtb_d[tch])
            nc.sync.dma_start(out=wk_full, in_=wk_d.rearrange("(c p) n -> p c n", p=128))
            nc.sync.dma_start(out=wv_full, in_=wv_d.rearrange("(c p) n -> p c n", p=128))
            nc.sync.dma_start(out=wd_sb, in_=wd_d.rearrange("(c p) n -> p c n", p=128))
            # w column of V: v_all[t, kc, h, 64] = w[kc*128+t] for every head
            for tb in range(NKC):
                nc.gpsimd.dma_start(
                    out=v_all[:, tb, :, 64:65],
                    in_=bass.AP(
                        tensor=wexpbf_d.tensor,
                        offset=wexpbf_d.offset + tb * 128,
                        ap=[[1, 128], [0, NH], [0, 1]],
                    ),
                )

            # --- phase 2: per head-pair projections + attention ---
            # (V build for all heads is emitted inside the hp==0 iteration so it
            #  overlaps head-pair 0's softmax on ScalarE.)
            with (
                tc.tile_pool(name="pairbuf", bufs=2) as pairbuf,
                tc.tile_pool(name="exps", bufs=8) as exps_pool,
                tc.tile_pool(name="sums", bufs=1) as sums_pool,
                tc.tile_pool(name="pp", bufs=2, space="PSUM") as pp,
                tc.tile_pool(name="sp", bufs=2, space="PSUM") as sp,
                tc.tile_pool(name="cp", bufs=2, space="PSUM") as cp,
            ):
                # V build for all heads (own phase, 128x128 mode only)
                for tb in range(NKC):
                    for nh in range(2):
                        pv = pp.tile([128, 512], fp32, tag="proj")
                        for ci in range(NCI):
                            nc.tensor.matmul(
                                pv,
                                xt[:, ci, tb * 128 : (tb + 1) * 128],
                                wv_full[:, ci, nh * 512 : (nh + 1) * 512],
                                start=(ci == 0),
                                stop=False,
                            )
                        nc.tensor.matmul(
                            pv,
                            ones1,
                            bv_row[:, nh * 512 : (nh + 1) * 512],
                            start=False,
                            stop=True,
                        )
                        # scale by w while evacuating PSUM
                        nc.vector.tensor_scalar_mul(
                            out=v_all[:, tb, nh * 8 : (nh + 1) * 8, 0:64],
                            in0=pv.rearrange("p (a b) -> p a b", a=8),
                            scalar1=wexp_sb[:, tb : tb + 1],
                        )
                for hp in range(NPAIR):
                    cols = slice(hp * 128, (hp + 1) * 128)
                    # Q'^T per head: [65, SQ] -- rows 0:64 = Q^T, row 64 = 0
                    # (the mask is folded into V via w, so row 64 just pads the
                    #  contraction to 65 to stay in the fast 128x128 PE mode)
                    qtp = [
                        pairbuf.tile([65, SQ], bf16, tag=f"qtp{h}", name=f"qtp{h}")
                        for h in range(2)
                    ]
                    for hl in range(2):
                        nc.gpsimd.memset(qtp[hl][64:65, :], 0.0)
                    for qb in range(SQ // 512):
                        pq = pp.tile([128, 512], fp32, tag="proj")
                        for ci in range(NCI):
                            nc.tensor.matmul(
                                pq,
                                wq_full[:, ci, cols],
                                xt[:, ci, qb * 512 : (qb + 1) * 512],
                                start=(ci == 0),
                                stop=(ci == NCI - 1),
                            )
                        for hl in range(2):
                            nc.vector.tensor_scalar_add(
                                out=qtp[hl][0:64, qb * 512 : (qb + 1) * 512],
                                in0=pq[hl * 64 : (hl + 1) * 64, :],
                                scalar1=bqT[hl * 64 : (hl + 1) * 64, hp : hp + 1],
                            )

                    # K'^T per head: [65, S] -- row 64 = 0
                    ktp = [
                        pairbuf.tile([65, S], bf16, tag=f"ktp{h}", name=f"ktp{h}")
                        for h in range(2)
                    ]
                    for hl in range(2):
                        nc.gpsimd.memset(ktp[hl][64:65, :], 0.0)
                    for tb in range(S // 512):
                        pk = pp.tile([128, 512], fp32, tag="proj")
                        for ci in range(NCI):
                            nc.tensor.matmul(
                                pk,
                                wk_full[:, ci, cols],
                                xt[:, ci, tb * 512 : (tb + 1) * 512],
                                start=(ci == 0),
                                stop=(ci == NCI - 1),
                            )
                        for hl in range(2):
                            nc.vector.tensor_scalar_add(
                                out=ktp[hl][0:64, tb * 512 : (tb + 1) * 512],
                                in0=pk[hl * 64 : (hl + 1) * 64, :],
                                scalar1=bkT[hl * 64 : (hl + 1) * 64, hp : hp + 1],
                            )

                    # attention for this pair
                    for qb in range(SQ // 512):
                        qsl = slice(qb * 512, (qb + 1) * 512)
                        # scores + exp, per key chunk, both heads at once
                        es_list = []
                        for kc in range(NKC):
                            ksl = slice(kc * 128, (kc + 1) * 128)
                            ps2 = sp.tile([128, 2, 512], fp32)
                            for hl in range(2):
                                nc.tensor.matmul(
                                    ps2[:, hl, :],
                                    ktp[hl][:, ksl],
                                    qtp[hl][:, qsl],
                                    start=True,
                                    stop=True,
                                )
                            es2 = exps_pool.tile([128, 2, 512], bf16)
                            nc.scalar.activation(
                                out=es2.rearrange("p a b -> p (a b)"),
                                in_=ps2.rearrange("p a b -> p (a b)"),
                                func=AF.Exp,
                                scale=0.125,
                            )
                            es_list.append(es2)


                        # ctx accumulation per head
                        pc = [
                            cp.tile([65, 512], fp32, tag=f"pc{hl}", name=f"pc{hl}", bufs=1)
                            for hl in range(2)
                        ]
                        for kc in range(NKC):
                            for hl in range(2):
                                nc.tensor.matmul(
                                    pc[hl],
                                    v_all[:, kc, hp * 2 + hl, :],
                                    es_list[kc][:, hl, :],
                                    start=(kc == 0),
                                    stop=(kc == NKC - 1),
                                )

                        # normalize: ctxt[:, hp, qsl] = ctx^T / den
                        for hl in range(2):
                            hsl = slice(hl * 64, (hl + 1) * 64)
                            nc.vector.tensor_copy(out=ctxt[hsl, hp, qsl], in_=pc[hl][0:64, :])
                        pb = pp.tile([128, 512], fp32, tag="proj")
                        for hl in range(2):
                            sums1 = sums_pool.tile(
                                [1, 512], fp32, tag=f"sums{hl}", name=f"sums{hl}"
                            )
                            nc.vector.tensor_copy(out=sums1, in_=pc[hl][64:65, :])
                            recip1 = sums_pool.tile([1, 512], fp32, tag=f"recip{hl}")
                            nc.vector.reciprocal_approx_fast(out=recip1, in_=sums1)
                            nc.vector.tensor_copy(
                                out=recip_bf[hl * 64 : hl * 64 + 1, :], in_=recip1
                            )
                        nc.tensor.matmul(pb, sel65, recip_bf, start=True, stop=True)
                        nc.vector.tensor_mul(ctxt[:, hp, qsl], ctxt[:, hp, qsl], pb)

            ctx_mid.__exit__(None, None, None)

            # --- phase 3: output projection + LayerNorm + residual ---
            with (
                tc.tile_pool(name="hid", bufs=3) as hid_pool,
                tc.tile_pool(name="lnbuf", bufs=3) as lnbuf,
                tc.tile_pool(name="op", bufs=3, space="PSUM") as op_pool,
            ):
                for qt_ in range(SQ // 128):
                    qsl = slice(qt_ * 128, (qt_ + 1) * 128)
                    hid = hid_pool.tile([128, H], fp32)
                    stats = lnbuf.tile([128, 2, 6], fp32, tag="stats")
                    for nb in range(2):
                        po = op_pool.tile([128, 512], fp32)
                        for ci in range(NCI):
                            nc.tensor.matmul(
                                po,
                                ctxt[:, ci, qsl],
                                wd_sb[:, ci, nb * 512 : (nb + 1) * 512],
                                start=(ci == 0),
                                stop=False,
                            )
                        nc.tensor.matmul(
                            po,
                            ones1,
                            bd_row[:, nb * 512 : (nb + 1) * 512],
                            start=False,
                            stop=True,
                        )
                        # LayerNorm stats straight off PSUM; ScalarE evacuates
                        nc.vector.bn_stats(out=stats[:, nb, :], in_=po)
                        nc.scalar.copy(out=hid[:, nb * 512 : (nb + 1) * 512], in_=po)
                    mv = lnbuf.tile([128, 2], fp32, tag="mv")
                    nc.vector.bn_aggr(out=mv, in_=stats)
                    rstd = lnbuf.tile([128, 1], fp32, tag="rstd")
                    nc.scalar.activation(
                        out=rstd, in_=mv[:, 1:2], func=AF.Sqrt, bias=eps_sb
                    )
                    nc.vector.reciprocal(rstd, rstd)
                    # residual + beta (overlaps with stats)
                    x_res = lnbuf.tile([128, H], fp32, tag="xres")
                    nc.sync.dma_start(out=x_res, in_=xkv[qsl, :])
                    xbeta = lnbuf.tile([128, H], fp32, tag="xbeta")
                    nc.gpsimd.tensor_tensor(out=xbeta, in0=x_res, in1=beta_b, op=OP.add)
                    # (hid - mu) * rstd * gamma + (x + beta)
                    norm = lnbuf.tile([128, H], fp32, tag="norm")
                    nc.vector.tensor_scalar(
                        out=norm,
                        in0=hid,
                        scalar1=mv[:, 0:1],
                        scalar2=rstd,
                        op0=OP.subtract,
                        op1=OP.mult,
                    )
                    nc.vector.tensor_mul(norm, norm, gamma_b)
                    final = lnbuf.tile([128, H], fp32, tag="final")
                    nc.gpsimd.tensor_tensor(out=final, in0=norm, in1=xbeta, op=OP.add)
                    nc.sync.dma_start(out=out_d[qsl, :], in_=final)

    nc.compile()
    return nc


def get_nc():
    if "nc" not in _cache:
        _cache["nc"] = _build()
    return _cache["nc"]


def make_in_maps(inputs):
    q = np.ascontiguousarray(np.asarray(inputs["query"], dtype=np.float32))
    am = np.asarray(inputs["attention_mask"], dtype=np.float32).reshape(B, S)
    import ml_dtypes

    bfl = ml_dtypes.bfloat16
    shared = {
        "wq": np.ascontiguousarray(np.asarray(inputs["Wq"], np.float32).astype(bfl)),
        "wk": np.ascontiguousarray(np.asarray(inputs["Wk"], np.float32).astype(bfl)),
        "wv": np.ascontiguousarray(np.asarray(inputs["Wv"], np.float32).astype(bfl)),
        "wd": np.ascontiguousarray(np.asarray(inputs["Wd"], np.float32).astype(bfl)),
        "bq": np.asarray(inputs["bq"], np.float32),
        "bk": np.asarray(inputs["bk"], np.float32),
        "bv": np.asarray(inputs["bv"], np.float32).astype(bfl),
        "bd": np.asarray(inputs["bd"], np.float32).astype(bfl),
        "gamma": np.asarray(inputs["ln_gamma"], np.float32),
        "beta": np.asarray(inputs["ln_beta"], np.float32),
    }
    in_maps = []
    for c in range(NCORES):
        b, hf = c // 2, c % 2
        # queries first, then the other half -- key order is softmax-invariant
        if hf == 0:
            xkv = q[b]
            mask = am[b]
        else:
            xkv = np.concatenate([q[b, SQ:], q[b, :SQ]], axis=0)
            mask = np.concatenate([am[b, SQ:], am[b, :SQ]], axis=0)
        m = dict(shared)
        m["xkv"] = np.ascontiguousarray(xkv)
        xtc = xkv.reshape(S // 128, 128, H // 128, 128).transpose(0, 3, 2, 1)
        m["xtb"] = np.ascontiguousarray(xtc.astype(bfl))
        wexp = np.exp(mask).astype(np.float32)
        m["wexp32"] = np.ascontiguousarray(wexp)
        m["wexpbf"] = np.ascontiguousarray(wexp.astype(bfl))
        in_maps.append(m)
    return in_maps


def assemble(results):
    out = np.empty((B, S, H), dtype=np.float32)
    for c in range(NCORES):
        b, hf = c // 2, c % 2
        out[b, hf * SQ : (hf + 1) * SQ, :] = results[c]["out"]
    return out


def kernel(**inputs):
    from concourse.bass_utils import run_bass_kernel_spmd

    nc = get_nc()
    in_maps = make_in_maps(inputs)
    res = run_bass_kernel_spmd(nc, in_maps, core_ids=list(range(NCORES)))
    return assemble(res.results)


if __name__ == "__main__":
    rng = np.random.default_rng(0)
    inputs = {
        "query": rng.standard_normal((B, S, H), dtype=np.float32),
        "attention_mask": np.zeros((B, 1, 1, S), np.float32),
        "Wq": rng.standard_normal((H, H), dtype=np.float32) * 0.02,
        "bq": np.zeros(H, np.float32),
        "Wk": rng.standard_normal((H, H), dtype=np.float32) * 0.02,
        "bk": np.zeros(H, np.float32),
        "Wv": rng.standard_normal((H, H), dtype=np.float32) * 0.02,
        "bv": np.zeros(H, np.float32),
        "Wd": rng.standard_normal((H, H), dtype=np.float32) * 0.02,
        "bd": np.zeros(H, np.float32),
        "ln_gamma": np.ones(H, np.float32),
        "ln_beta": np.zeros(H, np.float32),
    }
    out = kernel(**inputs)
    print(out.shape, out.dtype)


# revision 13
# speedup vs baseline: 1.2468x; 1.2427x over previous
"""Multi-head attention + output dense + LayerNorm + residual, on 8 NeuronCores.

Sharding: core c -> (batch b = c//2, query-half hf = c%2). Each core runs the
full 16-head attention for its 1024 queries against its batch's full 2048
keys (K/V projections are recomputed per query-half; no collectives needed).
The host reorders tokens so each core's queries are always rows 0:1024 of its
input slab -- key order is softmax-invariant as long as the mask is permuted
identically, so the device program is completely SPMD-uniform.

v2 layout choices (vs v1):
  - The additive attention mask is folded multiplicatively into V:
    softmax(S + m)_k = exp(S_k) w_k / sum_j exp(S_j) w_j with w = exp(m).
    V rows are scaled by w and the denominator column of V holds w, so the
    score matmuls have a pure 64-deep contraction (no mask row).
  - Score matmuls for the two heads of a pair run CONCURRENTLY on the PE
    array via 64x128 row tiling (tile_position (0,0)/(64,0)): K^T/Q^T for
    head-even live on SBUF partitions 0:64, head-odd on 64:128. This halves
    PE time for scores.
  - One exp activation per key chunk covers both heads ([128,2,512] PSUM ->
    bf16), with deep es buffering so the V-build phase overlaps head-pair
    0's softmax on ScalarE.
  - ctx matmuls keep the 65-column V (65th column = w) so row 64 of the
    accumulated ctx is the softmax denominator for free.
  - Phase 3 evacuates the out-proj PSUM via ScalarE copy (idle in the tail),
    runs bn_stats directly on PSUM, and puts residual adds on GpSimd.
"""

import numpy as np

B, S, H, NH = 4, 2048, 1024, 16
HD = H // NH  # 64
SQ = S // 2  # queries per core
NCORES = 8
NPAIR = NH // 2  # head pairs
NCI = H // 128  # 8 contraction chunks
NKC = S // 128  # 16 key chunks
EPS = 1e-12

_cache = {}


def _build():
    import concourse.bass as bass
    import concourse.bacc as bacc
    import concourse.mybir as mybir
    import concourse.tile as tile

    fp32 = mybir.dt.float32
    bf16 = mybir.dt.bfloat16
    AF = mybir.ActivationFunctionType
    OP = mybir.AluOpType

    nc = bacc.Bacc("TRN2", target_bir_lowering=False, debug=False)

    xkv = nc.dram_tensor("xkv", [S, H], fp32, kind="ExternalInput").ap()
    xtb_d = nc.dram_tensor("xtb", [NKC, 128, NCI, 128], bf16, kind="ExternalInput").ap()
    wexp32_d = nc.dram_tensor("wexp32", [S], fp32, kind="ExternalInput").ap()
    wexpbf_d = nc.dram_tensor("wexpbf", [S], bf16, kind="ExternalInput").ap()
    wq_d = nc.dram_tensor("wq", [H, H], bf16, kind="ExternalInput").ap()
    wk_d = nc.dram_tensor("wk", [H, H], bf16, kind="ExternalInput").ap()
    wv_d = nc.dram_tensor("wv", [H, H], bf16, kind="ExternalInput").ap()
    wd_d = nc.dram_tensor("wd", [H, H], bf16, kind="ExternalInput").ap()
    bq_d = nc.dram_tensor("bq", [H], fp32, kind="ExternalInput").ap()
    bk_d = nc.dram_tensor("bk", [H], fp32, kind="ExternalInput").ap()
    bv_d = nc.dram_tensor("bv", [H], bf16, kind="ExternalInput").ap()
    bd_d = nc.dram_tensor("bd", [H], bf16, kind="ExternalInput").ap()
    gamma_d = nc.dram_tensor("gamma", [H], fp32, kind="ExternalInput").ap()
    beta_d = nc.dram_tensor("beta", [H], fp32, kind="ExternalInput").ap()
    out_d = nc.dram_tensor("out", [SQ, H], fp32, kind="ExternalOutput").ap()

    def bcast128(ap):
        return bass.AP(tensor=ap.tensor, offset=ap.offset, ap=[[0, 128]] + list(ap.ap))

    def row1(ap):
        return bass.AP(tensor=ap.tensor, offset=ap.offset, ap=[[0, 1]] + list(ap.ap))

    with tile.TileContext(nc) as tc:
        with (
            tc.tile_pool(name="consts", bufs=1) as consts,
            tc.tile_pool(name="ctxT", bufs=1) as ctxt_pool,
        ):
            # --- constants ---
            bqT = consts.tile([128, NCI], fp32)
            nc.sync.dma_start(out=bqT, in_=bq_d.rearrange("(c p) -> p c", p=128))
            bkT = consts.tile([128, NCI], fp32)
            nc.sync.dma_start(out=bkT, in_=bk_d.rearrange("(c p) -> p c", p=128))
            wexp_sb = consts.tile([128, NKC], fp32)
            nc.sync.dma_start(out=wexp_sb, in_=wexp32_d.rearrange("(c p) -> p c", p=128))
            gamma_b = consts.tile([128, H], fp32)
            nc.sync.dma_start(out=gamma_b, in_=bcast128(gamma_d))
            beta_b = consts.tile([128, H], fp32)
            nc.sync.dma_start(out=beta_b, in_=bcast128(beta_d))
            eps_sb = consts.tile([128, 1], fp32)
            nc.vector.memset(eps_sb, EPS)
            sel65 = consts.tile([65, 128], bf16)
            nc.vector.memset(sel65, 0.0)
            nc.vector.memset(sel65[0:1, 0:64], 1.0)
            nc.vector.memset(sel65[64:65, 64:128], 1.0)
            recip_bf = consts.tile([65, 512], bf16)
            nc.vector.memset(recip_bf, 0.0)
            ones16 = consts.tile([128, NH], bf16)
            nc.vector.memset(ones16, 1.0)
            ones1 = consts.tile([1, 128], bf16)
            nc.vector.memset(ones1, 1.0)
            bv_row = consts.tile([1, H], bf16)
            nc.gpsimd.dma_start(out=bv_row, in_=row1(bv_d))
            bd_row = consts.tile([1, H], bf16)
            nc.gpsimd.dma_start(out=bd_row, in_=row1(bd_d))

            # ctxT[hl*64+d, hp, q] = ctx[q, (hp*2+hl)*64+d] / den
            ctxt = ctxt_pool.tile([128, NPAIR, SQ], bf16)
            wd_sb = ctxt_pool.tile([128, NCI, H], bf16, name="wd_sb")

            ctx_mid = tc.tile_pool(name="midA", bufs=1)
            midA = ctx_mid.__enter__()
            xt = midA.tile([128, NCI, S], bf16, name="xt")
            wq_full = midA.tile([128, NCI, H], bf16, name="wq_full")
            wk_full = midA.tile([128, NCI, H], bf16, name="wk_full")
            wv_full = midA.tile([128, NCI, H], bf16, name="wv_full")
            v_all = midA.tile([128, NKC, NH, 65], bf16, name="v_all")

            # input DMAs: what head-pair 0's projections need comes first
            nc.sync.dma_start(out=wq_full, in_=wq_d.rearrange("(c p) n -> p c n", p=128))
            for tch in range(NKC):
                nc.sync.dma_start(out=xt[:, :, tch * 128 : (tch + 1) * 128], in_=xtb_d[tch])
            nc.sync.dma_start(out=wk_full, in_=wk_d.rearrange("(c p) n -> p c n", p=128))
            nc.sync.dma_start(out=wv_full, in_=wv_d.rearrange("(c p) n -> p c n", p=128))
            nc.sync.dma_start(out=wd_sb, in_=wd_d.rearrange("(c p) n -> p c n", p=128))
            # w column of V: v_all[t, kc, h, 64] = w[kc*128+t] for every head
            # (computed on GpSimd from SBUF -- a broadcast DMA here is a
            #  descriptor storm that stalls the whole startup)
            for tb in range(NKC):
                nc.gpsimd.tensor_scalar_mul(
                    out=v_all[:, tb, :, 64:65].rearrange("p h one -> p (h one)"),
                    in0=ones16,
                    scalar1=wexp_sb[:, tb : tb + 1],
                )

            # --- phase 2: per head-pair projections + attention ---
            # Static software pipelining: V-build rides head-pair 0's first
            # kc loop (ctx(kc) needs only v_all[:, kc]); projections for
            # head-pair hp+1 are spread through hp's attention so the PE FIFO
            # always has independent work while ScalarE grinds exp.
            with (
                tc.tile_pool(name="pairbuf", bufs=2) as pairbuf,
                tc.tile_pool(name="exps", bufs=8) as exps_pool,
                tc.tile_pool(name="sums", bufs=1) as sums_pool,
                tc.tile_pool(name="pp", bufs=2, space="PSUM") as pp,
                tc.tile_pool(name="sp", bufs=2, space="PSUM") as sp,
                tc.tile_pool(name="cp", bufs=2, space="PSUM") as cp,
            ):
                qtp_t = {}
                ktp_t = {}

                def emit_vbuild(tb):
                    for nh in range(2):
                        pv = pp.tile([128, 512], fp32, tag="proj", name="pv")
                        for ci in range(NCI):
                            nc.tensor.matmul(
                                pv,
                                xt[:, ci, tb * 128 : (tb + 1) * 128],
                                wv_full[:, ci, nh * 512 : (nh + 1) * 512],
                                start=(ci == 0),
                                stop=False,
                            )
                        nc.tensor.matmul(
                            pv,
                            ones1,
                            bv_row[:, nh * 512 : (nh + 1) * 512],
                            start=False,
                            stop=True,
                        )
                        # scale by w while evacuating PSUM
                        nc.vector.tensor_scalar_mul(
                            out=v_all[:, tb, nh * 8 : (nh + 1) * 8, 0:64],
                            in0=pv.rearrange("p (a b) -> p a b", a=8),
                            scalar1=wexp_sb[:, tb : tb + 1],
                        )

                def emit_alloc_pair(hp):
                    qtp = [
                        pairbuf.tile([65, SQ], bf16, tag=f"qtp{h}", name=f"qtp{h}")
                        for h in range(2)
                    ]
                    ktp = [
                        pairbuf.tile([65, S], bf16, tag=f"ktp{h}", name=f"ktp{h}")
                        for h in range(2)
                    ]
                    for hl in range(2):
                        nc.gpsimd.memset(qtp[hl][64:65, :], 0.0)
                        nc.gpsimd.memset(ktp[hl][64:65, :], 0.0)
                    qtp_t[hp] = qtp
                    ktp_t[hp] = ktp

                def emit_qproj(hp, qb):
                    cols = slice(hp * 128, (hp + 1) * 128)
                    qtp = qtp_t[hp]
                    pq = pp.tile([128, 512], fp32, tag="proj", name="pq")
                    for ci in range(NCI):
                        nc.tensor.matmul(
                            pq,
                            wq_full[:, ci, cols],
                            xt[:, ci, qb * 512 : (qb + 1) * 512],
                            start=(ci == 0),
                            stop=(ci == NCI - 1),
                        )
                    for hl in range(2):
                        nc.vector.tensor_scalar_add(
                            out=qtp[hl][0:64, qb * 512 : (qb + 1) * 512],
                            in0=pq[hl * 64 : (hl + 1) * 64, :],
                            scalar1=bqT[hl * 64 : (hl + 1) * 64, hp : hp + 1],
                        )

                def emit_kproj(hp, tb):
                    cols = slice(hp * 128, (hp + 1) * 128)
                    ktp = ktp_t[hp]
                    pk = pp.tile([128, 512], fp32, tag="proj", name="pk")
                    for ci in range(NCI):
                        nc.tensor.matmul(
                            pk,
                            wk_full[:, ci, cols],
                            xt[:, ci, tb * 512 : (tb + 1) * 512],
                            start=(ci == 0),
                            stop=(ci == NCI - 1),
                        )
                    for hl in range(2):
                        nc.vector.tensor_scalar_add(
                            out=ktp[hl][0:64, tb * 512 : (tb + 1) * 512],
                            in0=pk[hl * 64 : (hl + 1) * 64, :],
                            scalar1=bkT[hl * 64 : (hl + 1) * 64, hp : hp + 1],
                        )

                def emit_attention_qb(hp, qb, extras):
                    qtp, ktp = qtp_t[hp], ktp_t[hp]
                    qsl = slice(qb * 512, (qb + 1) * 512)
                    pc = [
                        cp.tile([65, 512], fp32, tag=f"pc{hl}", name=f"pc{hl}", bufs=1)
                        for hl in range(2)
                    ]
                    for kc in range(NKC):
                        ksl = slice(kc * 128, (kc + 1) * 128)
                        ps2 = sp.tile([128, 2, 512], fp32, name="ps2")
                        for hl in range(2):
                            nc.tensor.matmul(
                                ps2[:, hl, :],
                                ktp[hl][:, ksl],
                                qtp[hl][:, qsl],
                                start=True,
                                stop=True,
                            )
                        es2 = exps_pool.tile([128, 2, 512], bf16, name="es2")
                        nc.scalar.activation(
                            out=es2.rearrange("p a b -> p (a b)"),
                            in_=ps2.rearrange("p a b -> p (a b)"),
                            func=AF.Exp,
                            scale=0.125,
                        )
                        for th in extras.get(kc, ()):
                            th()
                        for hl in range(2):
                            nc.tensor.matmul(
                                pc[hl],
                                v_all[:, kc, hp * 2 + hl, :],
                                es2[:, hl, :],
                                start=(kc == 0),
                                stop=(kc == NKC - 1),
                            )
                    # normalize: ctxt[:, hp, qsl] = ctx^T / den
                    for hl in range(2):
                        hsl = slice(hl * 64, (hl + 1) * 64)
                        nc.vector.tensor_copy(out=ctxt[hsl, hp, qsl], in_=pc[hl][0:64, :])
                    pb = pp.tile([128, 512], fp32, tag="proj", name="pb")
                    for hl in range(2):
                        sums1 = sums_pool.tile(
                            [1, 512], fp32, tag=f"sums{hl}", name=f"sums{hl}"
                        )
                        nc.vector.tensor_copy(out=sums1, in_=pc[hl][64:65, :])
                        recip1 = sums_pool.tile([1, 512], fp32, tag=f"recip{hl}")
                        nc.vector.reciprocal_approx_fast(out=recip1, in_=sums1)
                        nc.vector.tensor_copy(
                            out=recip_bf[hl * 64 : hl * 64 + 1, :], in_=recip1
                        )
                    nc.tensor.matmul(pb, sel65, recip_bf, start=True, stop=True)
                    nc.vector.tensor_mul(ctxt[:, hp, qsl], ctxt[:, hp, qsl], pb)

                emit_proj_all_first = True
                emit_alloc_pair(0)
                for qb in range(2):
                    emit_qproj(0, qb)
                for tb in range(4):
                    emit_kproj(0, tb)
                # hp0/qb0: V build rides the kc loop (ctx(kc) waits on v_all[kc])
                emit_attention_qb(
                    0, 0, {kc: (lambda tb=kc: emit_vbuild(tb),) for kc in range(NKC)}
                )
                for hp in range(NPAIR):
                    nxt = hp + 1
                    if hp > 0:
                        ext0 = {}
                        if nxt < NPAIR:
                            ext0 = {
                                0: (lambda n=nxt: emit_alloc_pair(n),),
                                4: (lambda n=nxt: emit_kproj(n, 0),),
                                9: (lambda n=nxt: emit_kproj(n, 1),),
                            }
                        emit_attention_qb(hp, 0, ext0)
                    ext1 = {}
                    if nxt < NPAIR:
                        if hp == 0:
                            ext1 = {
                                0: (lambda n=nxt: emit_alloc_pair(n),),
                                2: (lambda n=nxt: emit_kproj(n, 0),),
                                5: (lambda n=nxt: emit_kproj(n, 1),),
                                7: (lambda n=nxt: emit_qproj(n, 0),),
                                9: (lambda n=nxt: emit_kproj(n, 2),),
                                11: (lambda n=nxt: emit_qproj(n, 1),),
                                13: (lambda n=nxt: emit_kproj(n, 3),),
                            }
                        else:
                            ext1 = {
                                2: (lambda n=nxt: emit_qproj(n, 0),),
                                6: (lambda n=nxt: emit_kproj(n, 2),),
                                10: (lambda n=nxt: emit_qproj(n, 1),),
                                13: (lambda n=nxt: emit_kproj(n, 3),),
                            }
                    emit_attention_qb(hp, 1, ext1)

            ctx_mid.__exit__(None, None, None)

            # --- phase 3: output projection + LayerNorm + residual ---
            with (
                tc.tile_pool(name="hid", bufs=3) as hid_pool,
                tc.tile_pool(name="lnbuf", bufs=3) as lnbuf,
                tc.tile_pool(name="op", bufs=3, space="PSUM") as op_pool,
            ):
                for qt_ in range(SQ // 128):
                    qsl = slice(qt_ * 128, (qt_ + 1) * 128)
                    hid = hid_pool.tile([128, H], fp32)
                    stats = lnbuf.tile([128, 2, 6], fp32, tag="stats")
                    for nb in range(2):
                        po = op_pool.tile([128, 512], fp32)
                        for ci in range(NCI):
                            nc.tensor.matmul(
                                po,
                                ctxt[:, ci, qsl],
                                wd_sb[:, ci, nb * 512 : (nb + 1) * 512],
                                start=(ci == 0),
                                stop=False,
                            )
                        nc.tensor.matmul(
                            po,
                            ones1,
                            bd_row[:, nb * 512 : (nb + 1) * 512],
                            start=False,
                            stop=True,
                        )
                        # LayerNorm stats straight off PSUM; ScalarE evacuates
                        nc.vector.bn_stats(out=stats[:, nb, :], in_=po)
                        nc.scalar.copy(out=hid[:, nb * 512 : (nb + 1) * 512], in_=po)
                    mv = lnbuf.tile([128, 2], fp32, tag="mv")
                    nc.vector.bn_aggr(out=mv, in_=stats)
                    rstd = lnbuf.tile([128, 1], fp32, tag="rstd")
                    nc.scalar.activation(
                        out=rstd, in_=mv[:, 1:2], func=AF.Sqrt, bias=eps_sb
                    )
                    nc.vector.reciprocal(rstd, rstd)
                    # residual + beta (overlaps with stats)
                    x_res = lnbuf.tile([128, H], fp32, tag="xres")
                    nc.sync.dma_start(out=x_res, in_=xkv[qsl, :])
                    xbeta = lnbuf.tile([128, H], fp32, tag="xbeta")
                    nc.gpsimd.tensor_tensor(out=xbeta, in0=x_res, in1=beta_b, op=OP.add)
                    # (hid - mu) * rstd * gamma + (x + beta)
                    norm = lnbuf.tile([128, H], fp32, tag="norm")
                    nc.vector.tensor_scalar(
                        out=norm,
                        in0=hid,
                        scalar1=mv[:, 0:1],
                        scalar2=rstd,
                        op0=OP.subtract,
                        op1=OP.mult,
                    )
                    nc.vector.tensor_mul(norm, norm, gamma_b)
                    final = lnbuf.tile([128, H], fp32, tag="final")
                    nc.gpsimd.tensor_tensor(out=final, in0=norm, in1=xbeta, op=OP.add)
                    nc.sync.dma_start(out=out_d[qsl, :], in_=final)

    nc.compile()
    return nc


def get_nc():
    if "nc" not in _cache:
        _cache["nc"] = _build()
    return _cache["nc"]


def make_in_maps(inputs):
    q = np.ascontiguousarray(np.asarray(inputs["query"], dtype=np.float32))
    am = np.asarray(inputs["attention_mask"], dtype=np.float32).reshape(B, S)
    import ml_dtypes

    bfl = ml_dtypes.bfloat16
    shared = {
        "wq": np.ascontiguousarray(np.asarray(inputs["Wq"], np.float32).astype(bfl)),
        "wk": np.ascontiguousarray(np.asarray(inputs["Wk"], np.float32).astype(bfl)),
        "wv": np.ascontiguousarray(np.asarray(inputs["Wv"], np.float32).astype(bfl)),
        "wd": np.ascontiguousarray(np.asarray(inputs["Wd"], np.float32).astype(bfl)),
        "bq": np.asarray(inputs["bq"], np.float32),
        "bk": np.asarray(inputs["bk"], np.float32),
        "bv": np.asarray(inputs["bv"], np.float32).astype(bfl),
        "bd": np.asarray(inputs["bd"], np.float32).astype(bfl),
        "gamma": np.asarray(inputs["ln_gamma"], np.float32),
        "beta": np.asarray(inputs["ln_beta"], np.float32),
    }
    in_maps = []
    for c in range(NCORES):
        b, hf = c // 2, c % 2
        # queries first, then the other half -- key order is softmax-invariant
        if hf == 0:
            xkv = q[b]
            mask = am[b]
        else:
            xkv = np.concatenate([q[b, SQ:], q[b, :SQ]], axis=0)
            mask = np.concatenate([am[b, SQ:], am[b, :SQ]], axis=0)
        m = dict(shared)
        m["xkv"] = np.ascontiguousarray(xkv)
        xtc = xkv.reshape(S // 128, 128, H // 128, 128).transpose(0, 3, 2, 1)
        m["xtb"] = np.ascontiguousarray(xtc.astype(bfl))
        wexp = np.exp(mask).astype(np.float32)
        m["wexp32"] = np.ascontiguousarray(wexp)
        m["wexpbf"] = np.ascontiguousarray(wexp.astype(bfl))
        in_maps.append(m)
    return in_maps


def assemble(results):
    out = np.empty((B, S, H), dtype=np.float32)
    for c in range(NCORES):
        b, hf = c // 2, c % 2
        out[b, hf * SQ : (hf + 1) * SQ, :] = results[c]["out"]
    return out


def kernel(**inputs):
    from concourse.bass_utils import run_bass_kernel_spmd

    nc = get_nc()
    in_maps = make_in_maps(inputs)
    res = run_bass_kernel_spmd(nc, in_maps, core_ids=list(range(NCORES)))
    return assemble(res.results)


if __name__ == "__main__":
    rng = np.random.default_rng(0)
    inputs = {
        "query": rng.standard_normal((B, S, H), dtype=np.float32),
        "attention_mask": np.zeros((B, 1, 1, S), np.float32),
        "Wq": rng.standard_normal((H, H), dtype=np.float32) * 0.02,
        "bq": np.zeros(H, np.float32),
        "Wk": rng.standard_normal((H, H), dtype=np.float32) * 0.02,
        "bk": np.zeros(H, np.float32),
        "Wv": rng.standard_normal((H, H), dtype=np.float32) * 0.02,
        "bv": np.zeros(H, np.float32),
        "Wd": rng.standard_normal((H, H), dtype=np.float32) * 0.02,
        "bd": np.zeros(H, np.float32),
        "ln_gamma": np.ones(H, np.float32),
        "ln_beta": np.zeros(H, np.float32),
    }
    out = kernel(**inputs)
    print(out.shape, out.dtype)


# revision 16
# speedup vs baseline: 1.3003x; 1.0429x over previous
"""Multi-head attention + output dense + LayerNorm + residual, on 8 NeuronCores.

Sharding: core c -> (batch b = c//2, query-half hf = c%2). Each core runs the
full 16-head attention for its 1024 queries against its batch's full 2048
keys (K/V projections are recomputed per query-half; no collectives needed).
The host reorders tokens so each core's queries are always rows 0:1024 of its
input slab -- key order is softmax-invariant as long as the mask is permuted
identically, so the device program is completely SPMD-uniform.

v2 layout choices (vs v1):
  - The additive attention mask is folded multiplicatively into V:
    softmax(S + m)_k = exp(S_k) w_k / sum_j exp(S_j) w_j with w = exp(m).
    V rows are scaled by w and the denominator column of V holds w, so the
    score matmuls have a pure 64-deep contraction (no mask row).
  - Score matmuls for the two heads of a pair run CONCURRENTLY on the PE
    array via 64x128 row tiling (tile_position (0,0)/(64,0)): K^T/Q^T for
    head-even live on SBUF partitions 0:64, head-odd on 64:128. This halves
    PE time for scores.
  - One exp activation per key chunk covers both heads ([128,2,512] PSUM ->
    bf16), with deep es buffering so the V-build phase overlaps head-pair
    0's softmax on ScalarE.
  - ctx matmuls keep the 65-column V (65th column = w) so row 64 of the
    accumulated ctx is the softmax denominator for free.
  - Phase 3 evacuates the out-proj PSUM via ScalarE copy (idle in the tail),
    runs bn_stats directly on PSUM, and puts residual adds on GpSimd.
"""

import numpy as np

B, S, H, NH = 4, 2048, 1024, 16
HD = H // NH  # 64
SQ = S // 2  # queries per core
NCORES = 8
NPAIR = NH // 2  # head pairs
NCI = H // 128  # 8 contraction chunks
NKC = S // 128  # 16 key chunks
EPS = 1e-12

_cache = {}


def _build():
    import concourse.bass as bass
    import concourse.bacc as bacc
    import concourse.mybir as mybir
    import concourse.tile as tile

    fp32 = mybir.dt.float32
    bf16 = mybir.dt.bfloat16
    fp8 = mybir.dt.float8e4
    AF = mybir.ActivationFunctionType
    OP = mybir.AluOpType

    nc = bacc.Bacc("TRN2", target_bir_lowering=False, debug=False)

    xkv = nc.dram_tensor("xkv", [S, H], fp32, kind="ExternalInput").ap()
    xtb_d = nc.dram_tensor("xtb", [NKC, 128, NCI, 128], bf16, kind="ExternalInput").ap()
    wexp32_d = nc.dram_tensor("wexp32", [S], fp32, kind="ExternalInput").ap()
    wexpbf_d = nc.dram_tensor("wexpbf", [S], bf16, kind="ExternalInput").ap()
    wq_d = nc.dram_tensor("wq", [H, H], bf16, kind="ExternalInput").ap()
    wk_d = nc.dram_tensor("wk", [H, H], bf16, kind="ExternalInput").ap()
    wv_d = nc.dram_tensor("wv", [H, H], bf16, kind="ExternalInput").ap()
    wd_d = nc.dram_tensor("wd", [H, H], bf16, kind="ExternalInput").ap()
    bq_d = nc.dram_tensor("bq", [H], fp32, kind="ExternalInput").ap()
    bk_d = nc.dram_tensor("bk", [H], fp32, kind="ExternalInput").ap()
    bv_d = nc.dram_tensor("bv", [H], bf16, kind="ExternalInput").ap()
    bd_d = nc.dram_tensor("bd", [H], bf16, kind="ExternalInput").ap()
    gamma_d = nc.dram_tensor("gamma", [H], fp32, kind="ExternalInput").ap()
    beta_d = nc.dram_tensor("beta", [H], fp32, kind="ExternalInput").ap()
    out_d = nc.dram_tensor("out", [SQ, H], fp32, kind="ExternalOutput").ap()

    def bcast128(ap):
        return bass.AP(tensor=ap.tensor, offset=ap.offset, ap=[[0, 128]] + list(ap.ap))

    def row1(ap):
        return bass.AP(tensor=ap.tensor, offset=ap.offset, ap=[[0, 1]] + list(ap.ap))

    with tile.TileContext(nc) as tc:
        with (
            tc.tile_pool(name="consts", bufs=1) as consts,
            tc.tile_pool(name="ctxT", bufs=1) as ctxt_pool,
        ):
            # --- constants ---
            bqT = consts.tile([128, NCI], fp32)
            nc.sync.dma_start(out=bqT, in_=bq_d.rearrange("(c p) -> p c", p=128))
            bkT = consts.tile([128, NCI], fp32)
            nc.sync.dma_start(out=bkT, in_=bk_d.rearrange("(c p) -> p c", p=128))
            wexp_sb = consts.tile([128, NKC], fp32)
            nc.sync.dma_start(out=wexp_sb, in_=wexp32_d.rearrange("(c p) -> p c", p=128))
            gamma_b = consts.tile([128, H], fp32)
            nc.sync.dma_start(out=gamma_b, in_=bcast128(gamma_d))
            beta_b = consts.tile([128, H], fp32)
            nc.sync.dma_start(out=beta_b, in_=bcast128(beta_d))
            eps_sb = consts.tile([128, 1], fp32)
            nc.vector.memset(eps_sb, EPS)
            sel65 = consts.tile([65, 128], bf16)
            nc.vector.memset(sel65, 0.0)
            nc.vector.memset(sel65[0:1, 0:64], 1.0)
            nc.vector.memset(sel65[64:65, 64:128], 1.0)
            recip_bf = consts.tile([65, 512], bf16)
            nc.vector.memset(recip_bf, 0.0)
            ones16 = consts.tile([128, NH], bf16)
            nc.vector.memset(ones16, 1.0)
            ones1 = consts.tile([1, 128], bf16)
            nc.vector.memset(ones1, 1.0)
            bv_row = consts.tile([1, H], bf16)
            nc.gpsimd.dma_start(out=bv_row, in_=row1(bv_d))
            bd_row = consts.tile([1, H], bf16)
            nc.gpsimd.dma_start(out=bd_row, in_=row1(bd_d))

            # ctxT[hl*64+d, hp, q] = ctx[q, (hp*2+hl)*64+d] / den
            ctxt = ctxt_pool.tile([128, NPAIR, SQ], bf16)
            wd_sb = ctxt_pool.tile([128, NCI, H], bf16, name="wd_sb")

            ctx_mid = tc.tile_pool(name="midA", bufs=1)
            midA = ctx_mid.__enter__()
            xt = midA.tile([128, NCI, S], bf16, name="xt")
            wq_full = midA.tile([128, NCI, H], bf16, name="wq_full")
            wk_full = midA.tile([128, NCI, H], bf16, name="wk_full")
            wv_full = midA.tile([128, NCI, H], bf16, name="wv_full")
            v8 = midA.tile([128, NKC // 2, 2, NH, 80], fp8, name="v8")

            # input DMAs, spread across engine queues so transfers parallelize:
            # hp0's Q/K proj needs wq/wk + the first xt chunks first
            nc.sync.dma_start(out=wq_full, in_=wq_d.rearrange("(c p) n -> p c n", p=128))
            nc.scalar.dma_start(out=wk_full, in_=wk_d.rearrange("(c p) n -> p c n", p=128))
            for tch in range(NKC):
                eng = (nc.sync, nc.scalar, nc.gpsimd)[tch % 3]
                eng.dma_start(out=xt[:, :, tch * 128 : (tch + 1) * 128], in_=xtb_d[tch])
            nc.scalar.dma_start(out=wv_full, in_=wv_d.rearrange("(c p) n -> p c n", p=128))
            nc.sync.dma_start(out=wd_sb, in_=wd_d.rearrange("(c p) n -> p c n", p=128))
            # w column of V: v_all[t, kc, h, 64] = w[kc*128+t] for every head
            # (computed on GpSimd from SBUF -- a broadcast DMA here is a
            #  descriptor storm that stalls the whole startup)
            for tb in range(NKC):
                nc.gpsimd.tensor_scalar_mul(
                    out=v8[:, tb // 2, tb % 2, :, 64:65].rearrange("p h one -> p (h one)"),
                    in0=ones16,
                    scalar1=wexp_sb[:, tb : tb + 1],
                )

            # --- phase 2: per head-pair projections + attention ---
            # Static software pipelining: V-build rides head-pair 0's first
            # kc loop (ctx(kc) needs only v_all[:, kc]); projections for
            # head-pair hp+1 are spread through hp's attention so the PE FIFO
            # always has independent work while ScalarE grinds exp.
            with (
                tc.tile_pool(name="pairbuf", bufs=2) as pairbuf,
                tc.tile_pool(name="exps", bufs=8) as exps_pool,
                tc.tile_pool(name="sums", bufs=1) as sums_pool,
                tc.tile_pool(name="pp", bufs=2, space="PSUM") as pp,
                tc.tile_pool(name="sp", bufs=2, space="PSUM") as sp,
                tc.tile_pool(name="cp", bufs=2, space="PSUM") as cp,
            ):
                qtp_t = {}
                ktp_t = {}

                def emit_vbuild(tb):
                    for nh in range(2):
                        pv = pp.tile([128, 512], fp32, tag="proj", name="pv")
                        for ci in range(NCI):
                            nc.tensor.matmul(
                                pv,
                                xt[:, ci, tb * 128 : (tb + 1) * 128],
                                wv_full[:, ci, nh * 512 : (nh + 1) * 512],
                                start=(ci == 0),
                                stop=False,
                            )
                        nc.tensor.matmul(
                            pv,
                            ones1,
                            bv_row[:, nh * 512 : (nh + 1) * 512],
                            start=False,
                            stop=True,
                        )
                        # scale by w while evacuating PSUM
                        nc.vector.tensor_scalar_mul(
                            out=v8[:, tb // 2, tb % 2, nh * 8 : (nh + 1) * 8, 0:64],
                            in0=pv.rearrange("p (a b) -> p a b", a=8),
                            scalar1=wexp_sb[:, tb : tb + 1],
                        )

                def emit_alloc_pair(hp):
                    qtp = [
                        pairbuf.tile([65, SQ], bf16, tag=f"qtp{h}", name=f"qtp{h}")
                        for h in range(2)
                    ]
                    ktp = [
                        pairbuf.tile([65, S], bf16, tag=f"ktp{h}", name=f"ktp{h}")
                        for h in range(2)
                    ]
                    for hl in range(2):
                        nc.gpsimd.memset(qtp[hl][64:65, :], 0.0)
                        nc.gpsimd.memset(ktp[hl][64:65, :], 0.0)
                    qtp_t[hp] = qtp
                    ktp_t[hp] = ktp

                def emit_qproj(hp, qb):
                    cols = slice(hp * 128, (hp + 1) * 128)
                    qtp = qtp_t[hp]
                    pq = pp.tile([128, 512], fp32, tag="proj", name="pq")
                    for ci in range(NCI):
                        nc.tensor.matmul(
                            pq,
                            wq_full[:, ci, cols],
                            xt[:, ci, qb * 512 : (qb + 1) * 512],
                            start=(ci == 0),
                            stop=(ci == NCI - 1),
                        )
                    for hl in range(2):
                        nc.vector.tensor_scalar_add(
                            out=qtp[hl][0:64, qb * 512 : (qb + 1) * 512],
                            in0=pq[hl * 64 : (hl + 1) * 64, :],
                            scalar1=bqT[hl * 64 : (hl + 1) * 64, hp : hp + 1],
                        )

                def emit_kproj(hp, tb):
                    cols = slice(hp * 128, (hp + 1) * 128)
                    ktp = ktp_t[hp]
                    pk = pp.tile([128, 512], fp32, tag="proj", name="pk")
                    for ci in range(NCI):
                        nc.tensor.matmul(
                            pk,
                            wk_full[:, ci, cols],
                            xt[:, ci, tb * 512 : (tb + 1) * 512],
                            start=(ci == 0),
                            stop=(ci == NCI - 1),
                        )
                    for hl in range(2):
                        nc.vector.tensor_scalar_add(
                            out=ktp[hl][0:64, tb * 512 : (tb + 1) * 512],
                            in0=pk[hl * 64 : (hl + 1) * 64, :],
                            scalar1=bkT[hl * 64 : (hl + 1) * 64, hp : hp + 1],
                        )

                def emit_attention_qb(hp, qb, extras):
                    qtp, ktp = qtp_t[hp], ktp_t[hp]
                    qsl = slice(qb * 512, (qb + 1) * 512)
                    pc = [
                        cp.tile([65, 512], fp32, tag=f"pc{hl}", name=f"pc{hl}", bufs=1)
                        for hl in range(2)
                    ]
                    for kj in range(NKC // 2):
                        es8 = []
                        for hl in range(2):
                            ps2 = sp.tile([128, 2, 512], fp32, name="ps2")
                            for j in range(2):
                                kc = kj * 2 + j
                                nc.tensor.matmul(
                                    ps2[:, j, :],
                                    ktp[hl][:, kc * 128 : (kc + 1) * 128],
                                    qtp[hl][:, qsl],
                                    start=True,
                                    stop=True,
                                )
                            e8 = exps_pool.tile([128, 2, 512], fp8, name="es8")
                            nc.scalar.activation(
                                out=e8.rearrange("p a b -> p (a b)"),
                                in_=ps2.rearrange("p a b -> p (a b)"),
                                func=AF.Exp,
                                scale=0.125,
                            )
                            es8.append(e8)
                        for th in extras.get(kj, ()):
                            th()
                        for hl in range(2):
                            nc.tensor.matmul(
                                pc[hl],
                                v8[:, kj, :, hp * 2 + hl, 0:65],
                                es8[hl],
                                start=(kj == 0),
                                stop=(kj == NKC // 2 - 1),
                                perf_mode=mybir.MatmulPerfMode.DoubleRow,
                            )
                    # normalize: ctxt[:, hp, qsl] = ctx^T / den
                    for hl in range(2):
                        hsl = slice(hl * 64, (hl + 1) * 64)
                        nc.vector.tensor_copy(out=ctxt[hsl, hp, qsl], in_=pc[hl][0:64, :])
                    pb = pp.tile([128, 512], fp32, tag="proj", name="pb")
                    for hl in range(2):
                        sums1 = sums_pool.tile(
                            [1, 512], fp32, tag=f"sums{hl}", name=f"sums{hl}"
                        )
                        nc.vector.tensor_copy(out=sums1, in_=pc[hl][64:65, :])
                        recip1 = sums_pool.tile([1, 512], fp32, tag=f"recip{hl}")
                        nc.vector.reciprocal_approx_fast(out=recip1, in_=sums1)
                        nc.vector.tensor_copy(
                            out=recip_bf[hl * 64 : hl * 64 + 1, :], in_=recip1
                        )
                    nc.tensor.matmul(pb, sel65, recip_bf, start=True, stop=True)
                    nc.vector.tensor_mul(ctxt[:, hp, qsl], ctxt[:, hp, qsl], pb)

                emit_proj_all_first = True
                emit_alloc_pair(0)
                for qb in range(2):
                    emit_qproj(0, qb)
                for tb in range(4):
                    emit_kproj(0, tb)
                # hp0/qb0: V build rides the kc loop (ctx(kc) waits on v_all[kc])
                emit_attention_qb(
                    0,
                    0,
                    {
                        kj: (
                            lambda tb=2 * kj: emit_vbuild(tb),
                            lambda tb=2 * kj + 1: emit_vbuild(tb),
                        )
                        for kj in range(NKC // 2)
                    },
                )
                for hp in range(NPAIR):
                    nxt = hp + 1
                    if hp > 0:
                        ext0 = {}
                        if nxt < NPAIR:
                            ext0 = {
                                0: (lambda n=nxt: emit_alloc_pair(n),),
                                2: (lambda n=nxt: emit_kproj(n, 0),),
                                5: (lambda n=nxt: emit_kproj(n, 1),),
                            }
                        emit_attention_qb(hp, 0, ext0)
                    ext1 = {}
                    if nxt < NPAIR:
                        if hp == 0:
                            ext1 = {
                                0: (lambda n=nxt: emit_alloc_pair(n),),
                                1: (lambda n=nxt: emit_kproj(n, 0),),
                                2: (lambda n=nxt: emit_kproj(n, 1),),
                                3: (lambda n=nxt: emit_qproj(n, 0),),
                                4: (lambda n=nxt: emit_kproj(n, 2),),
                                5: (lambda n=nxt: emit_qproj(n, 1),),
                                6: (lambda n=nxt: emit_kproj(n, 3),),
                            }
                        else:
                            ext1 = {
                                1: (lambda n=nxt: emit_qproj(n, 0),),
                                3: (lambda n=nxt: emit_kproj(n, 2),),
                                5: (lambda n=nxt: emit_qproj(n, 1),),
                                6: (lambda n=nxt: emit_kproj(n, 3),),
                            }
                    emit_attention_qb(hp, 1, ext1)

            ctx_mid.__exit__(None, None, None)

            # --- phase 3: output projection + LayerNorm + residual ---
            with (
                tc.tile_pool(name="hid", bufs=3) as hid_pool,
                tc.tile_pool(name="lnbuf", bufs=3) as lnbuf,
                tc.tile_pool(name="op", bufs=3, space="PSUM") as op_pool,
            ):
                for qt_ in range(SQ // 128):
                    qsl = slice(qt_ * 128, (qt_ + 1) * 128)
                    hid = hid_pool.tile([128, H], fp32)
                    stats = lnbuf.tile([128, 2, 6], fp32, tag="stats")
                    for nb in range(2):
                        po = op_pool.tile([128, 512], fp32)
                        for ci in range(NCI):
                            nc.tensor.matmul(
                                po,
                                ctxt[:, ci, qsl],
                                wd_sb[:, ci, nb * 512 : (nb + 1) * 512],
                                start=(ci == 0),
                                stop=False,
                            )
                        nc.tensor.matmul(
                            po,
                            ones1,
                            bd_row[:, nb * 512 : (nb + 1) * 512],
                            start=False,
                            stop=True,
                        )
                        # LayerNorm stats straight off PSUM; ScalarE evacuates
                        nc.vector.bn_stats(out=stats[:, nb, :], in_=po)
                        nc.scalar.copy(out=hid[:, nb * 512 : (nb + 1) * 512], in_=po)
                    mv = lnbuf.tile([128, 2], fp32, tag="mv")
                    nc.vector.bn_aggr(out=mv, in_=stats)
                    rstd = lnbuf.tile([128, 1], fp32, tag="rstd")
                    nc.scalar.activation(
                        out=rstd, in_=mv[:, 1:2], func=AF.Sqrt, bias=eps_sb
                    )
                    nc.vector.reciprocal(rstd, rstd)
                    # residual + beta (overlaps with stats)
                    x_res = lnbuf.tile([128, H], fp32, tag="xres")
                    nc.sync.dma_start(out=x_res, in_=xkv[qsl, :])
                    xbeta = lnbuf.tile([128, H], fp32, tag="xbeta")
                    nc.gpsimd.tensor_tensor(out=xbeta, in0=x_res, in1=beta_b, op=OP.add)
                    # (hid - mu) * rstd * gamma + (x + beta)
                    norm = lnbuf.tile([128, H], fp32, tag="norm")
                    nc.vector.tensor_scalar(
                        out=norm,
                        in0=hid,
                        scalar1=mv[:, 0:1],
                        scalar2=rstd,
                        op0=OP.subtract,
                        op1=OP.mult,
                    )
                    nc.vector.tensor_mul(norm, norm, gamma_b)
                    final = lnbuf.tile([128, H], fp32, tag="final")
                    nc.gpsimd.tensor_tensor(out=final, in0=norm, in1=xbeta, op=OP.add)
                    nc.sync.dma_start(out=out_d[qsl, :], in_=final)

    nc.compile()
    return nc


def get_nc():
    if "nc" not in _cache:
        _cache["nc"] = _build()
    return _cache["nc"]


def make_in_maps(inputs):
    q = np.ascontiguousarray(np.asarray(inputs["query"], dtype=np.float32))
    am = np.asarray(inputs["attention_mask"], dtype=np.float32).reshape(B, S)
    import ml_dtypes

    bfl = ml_dtypes.bfloat16
    shared = {
        "wq": np.ascontiguousarray(np.asarray(inputs["Wq"], np.float32).astype(bfl)),
        "wk": np.ascontiguousarray(np.asarray(inputs["Wk"], np.float32).astype(bfl)),
        "wv": np.ascontiguousarray(np.asarray(inputs["Wv"], np.float32).astype(bfl)),
        "wd": np.ascontiguousarray(np.asarray(inputs["Wd"], np.float32).astype(bfl)),
        "bq": np.asarray(inputs["bq"], np.float32),
        "bk": np.asarray(inputs["bk"], np.float32),
        "bv": np.asarray(inputs["bv"], np.float32).astype(bfl),
        "bd": np.asarray(inputs["bd"], np.float32).astype(bfl),
        "gamma": np.asarray(inputs["ln_gamma"], np.float32),
        "beta": np.asarray(inputs["ln_beta"], np.float32),
    }
    in_maps = []
    for c in range(NCORES):
        b, hf = c // 2, c % 2
        # queries first, then the other half -- key order is softmax-invariant
        if hf == 0:
            xkv = q[b]
            mask = am[b]
        else:
            xkv = np.concatenate([q[b, SQ:], q[b, :SQ]], axis=0)
            mask = np.concatenate([am[b, SQ:], am[b, :SQ]], axis=0)
        m = dict(shared)
        m["xkv"] = np.ascontiguousarray(xkv)
        xtc = xkv.reshape(S // 128, 128, H // 128, 128).transpose(0, 3, 2, 1)
        m["xtb"] = np.ascontiguousarray(xtc.astype(bfl))
        wexp = np.exp(mask).astype(np.float32)
        m["wexp32"] = np.ascontiguousarray(wexp)
        m["wexpbf"] = np.ascontiguousarray(wexp.astype(bfl))
        in_maps.append(m)
    return in_maps


def assemble(results):
    out = np.empty((B, S, H), dtype=np.float32)
    for c in range(NCORES):
        b, hf = c // 2, c % 2
        out[b, hf * SQ : (hf + 1) * SQ, :] = results[c]["out"]
    return out


def kernel(**inputs):
    from concourse.bass_utils import run_bass_kernel_spmd

    nc = get_nc()
    in_maps = make_in_maps(inputs)
    res = run_bass_kernel_spmd(nc, in_maps, core_ids=list(range(NCORES)))
    return assemble(res.results)


if __name__ == "__main__":
    rng = np.random.default_rng(0)
    inputs = {
        "query": rng.standard_normal((B, S, H), dtype=np.float32),
        "attention_mask": np.zeros((B, 1, 1, S), np.float32),
        "Wq": rng.standard_normal((H, H), dtype=np.float32) * 0.02,
        "bq": np.zeros(H, np.float32),
        "Wk": rng.standard_normal((H, H), dtype=np.float32) * 0.02,
        "bk": np.zeros(H, np.float32),
        "Wv": rng.standard_normal((H, H), dtype=np.float32) * 0.02,
        "bv": np.zeros(H, np.float32),
        "Wd": rng.standard_normal((H, H), dtype=np.float32) * 0.02,
        "bd": np.zeros(H, np.float32),
        "ln_gamma": np.ones(H, np.float32),
        "ln_beta": np.zeros(H, np.float32),
    }
    out = kernel(**inputs)
    print(out.shape, out.dtype)
